# revision 32
# baseline (speedup 1.0000x reference)
# Distributed causal multi-head attention for 8 TRN2 NeuronCores.
#
# Problem: B=1, S=4096, D=768, H=12 heads, d_head=64, fp32 I/O.
#   out = softmax(causal((x_q Wq^T)(x_k Wk^T)^T / 8)) (x_v Wv^T) Wo^T  (+biases)
#
# Sharding: sequence-parallel over queries.  The 32 q-tiles (128 rows each)
# are dealt to cores with stride 8: core i owns q-tiles {i, 8+i, 16+i, 24+i}
# ("slots" j=0..3).  To keep the graph SPMD-identical across cores, slot j
# statically processes n_j = 8j+8 key-tiles; keys beyond a core's causal
# limit are zeroed in the exp domain with a per-core 0/1 mask (data, not
# structure).  K/V projections are computed on each core for its own 512-row
# key chunk and shared via one AllGather.
#
# Score layout is transposed ([keys, q]) so that the softmax denominator
# falls out of the PV matmul via an appended ones-column on V, and the
# PV result lands directly in the [feature, q] layout that the output
# projection consumes as lhsT.

import numpy as np
import ml_dtypes

import concourse.bass as bass
import concourse.mybir as mybir
import concourse.tile as tile
from concourse import bacc
from concourse import bass_utils

P = 128
S = 4096
D = 768
H = 12
DH = 64
N_CORES = 8
NQ = 512                      # q rows per core
N_SLOTS = 4                   # q-tiles per core
SLOT_NKT = [8, 16, 24, 32]    # key tiles processed per slot (static)
TOT_NKT = sum(SLOT_NKT)       # 80
S_LOC = S // N_CORES          # 512 keys projected per core
KT_LOC = S_LOC // P           # 4 key tiles per core chunk
VW = DH + 1                   # v columns per head incl. ones column (65)
EXP_BATCH = 12                # key tiles per PSUM batch for exp
F32 = mybir.dt.float32
BF16 = mybir.dt.bfloat16
NEG = -1e9
DEBUG = False


def q_tiles_of_core(i):
    return [8 * j + i for j in range(N_SLOTS)]


def build(has_bias: bool):
    kt_in = 7 if has_bias else 6      # input-feature contraction tiles
    d_in = kt_in * P                  # 768 or 896 (ones row + zero pad)

    nc = bacc.Bacc("TRN2", target_bir_lowering=False, debug=False,
                   num_devices=N_CORES)

    # ---- I/O ----
    # all inputs host-prearranged to [128 partitions, contiguous free]
    qT_d = nc.dram_tensor("qT", [P, kt_in * NQ], BF16, kind="ExternalInput")
    kT_d = nc.dram_tensor("kT", [P, kt_in * S], BF16,
                          kind="ExternalInput")
    vT_d = nc.dram_tensor("vT", [P, kt_in * S], BF16,
                          kind="ExternalInput")
    wqT_d = nc.dram_tensor("wqT", [P, kt_in * D], BF16, kind="ExternalInput")
    wkT_d = nc.dram_tensor("wkT", [P, kt_in * D], BF16, kind="ExternalInput")
    wvT_d = nc.dram_tensor("wvT", [P, kt_in * D], BF16, kind="ExternalInput")
    ho = H + 1 if has_bias else H     # wo contraction groups of 64
    woT_d = nc.dram_tensor("woT", [DH, ho * D], BF16, kind="ExternalInput")
    # static upper-triangular tile (valid iff key_row <= q_col)
    mask_d = nc.dram_tensor("mask", [P, P], BF16, kind="ExternalInput")
    out_d = nc.dram_tensor("out", [NQ, D], F32, kind="ExternalOutput")
    dbg = {}
    if DEBUG:
        dbg["qhT"] = nc.dram_tensor("dbg_qhT", [P, 6, NQ], BF16,
                                    kind="ExternalOutput")
        dbg["khT"] = nc.dram_tensor("dbg_khT", [P, 6, S], BF16,
                                    kind="ExternalOutput")
        dbg["vh"] = nc.dram_tensor("dbg_vh", [P, S // P, H * VW], BF16,
                                   kind="ExternalOutput")
        dbg["exp"] = nc.dram_tensor("dbg_exp", [P, SLOT_NKT[0] * P], BF16,
                                    kind="ExternalOutput")
        dbg["po"] = nc.dram_tensor("dbg_po", [VW, P], F32,
                                   kind="ExternalOutput")
        dbg["nrm"] = nc.dram_tensor("dbg_nrm", [3, P], F32,
                                    kind="ExternalOutput")
        dbg["aT"] = nc.dram_tensor("dbg_aT", [DH, H, NQ], BF16,
                                   kind="ExternalOutput")



    with tile.TileContext(nc) as tc:
        _body(nc, tc, locals(), has_bias, kt_in)

    nc.compile()
    return nc


def _body(nc, tc, t_, has_bias, kt_in):
    qT_d, kT_d, vT_d = t_["qT_d"], t_["kT_d"], t_["vT_d"]
    wqT_d, wkT_d, wvT_d, woT_d = (t_["wqT_d"], t_["wkT_d"], t_["wvT_d"],
                                  t_["woT_d"])
    mask_d, out_d = t_["mask_d"], t_["out_d"]
    dbg = t_["dbg"]

    ho = H + 1 if has_bias else H
    from contextlib import ExitStack
    ctx = ExitStack()
    with ctx:
        # persistent tensors (live through attention)
        big = ctx.enter_context(tc.tile_pool(name="big", bufs=1))
        psA = ctx.enter_context(
            tc.tile_pool(name="psA", bufs=2, space="PSUM"))   # wide groups
        psB = ctx.enter_context(
            tc.tile_pool(name="psB", bufs=2, space="PSUM"))   # PV accums

        qhT = big.tile([P, 6, NQ], BF16, tag="qhT")
        khT = big.tile([P, 6, S], BF16, tag="khT")
        vh = big.tile([P, S // P, H * VW], BF16, tag="vh")
        aT = big.tile([DH, ho, NQ], BF16, tag="aT")

        # ---------- projections (fully local, no collective) ----------
        with tc.tile_pool(name="pw", bufs=1) as pw, \
             tc.tile_pool(name="st", bufs=2) as st:
            wk_sb = pw.tile([P, kt_in, D], BF16, tag="wk")
            wv_sb = pw.tile([P, kt_in, D], BF16, tag="wv")
            nc.scalar.dma_start(wk_sb[:], wkT_d.ap().rearrange(
                "p (kt f) -> p kt f", f=D))
            nc.scalar.dma_start(wv_sb[:], wvT_d.ap().rearrange(
                "p (kt f) -> p kt f", f=D))

            # K/V windows interleaved: khT copies on ScalarE, vh on VectorE
            for sc in range(N_CORES):
                xk = st.tile([P, kt_in, S_LOC], BF16, tag="x", name="xk")
                nc.sync.dma_start(
                    xk[:],
                    kT_d.ap().rearrange("p (kt s) -> p kt s", s=S)
                    [:, :, sc * S_LOC:(sc + 1) * S_LOC])
                for ft in range(6):
                    ps = psA.tile([P, EXP_BATCH * P], F32, tag="scores")
                    for kt in range(kt_in):
                        nc.tensor.matmul(
                            ps[:, :S_LOC],
                            wk_sb[:, kt, ft * P:(ft + 1) * P],
                            xk[:, kt, :],
                            start=(kt == 0), stop=(kt == kt_in - 1))
                    nc.scalar.copy(
                        khT[:, ft, sc * S_LOC:(sc + 1) * S_LOC],
                        ps[:, :S_LOC])

                xv = st.tile([P, kt_in, S_LOC], BF16, tag="x", name="xv")
                nc.sync.dma_start(
                    xv[:],
                    vT_d.ap().rearrange("p (kt s) -> p kt s", s=S)
                    [:, :, sc * S_LOC:(sc + 1) * S_LOC])
                for stile in range(KT_LOC):
                    gkt = sc * KT_LOC + stile
                    for half in range(2):
                        ps = psA.tile([P, EXP_BATCH * P], F32, tag="scores")
                        for kt in range(kt_in):
                            nc.tensor.matmul(
                                ps[:, :384],
                                xv[:, kt, stile * P:(stile + 1) * P],
                                wv_sb[:, kt, half * 384:(half + 1) * 384],
                                start=(kt == 0), stop=(kt == kt_in - 1))
                        nc.vector.tensor_copy(
                            vh[:, gkt, half * 6 * VW:(half + 1) * 6 * VW]
                            .rearrange("p (hh w) -> p hh w", w=VW)[:, :, :DH],
                            ps[:, :384].rearrange("p (hh w) -> p hh w", w=DH))
            nc.vector.memset(
                vh[:].rearrange("p s (h w) -> p s h w", w=VW)
                [:, :, :, DH:], 1.0)

            # Q projection: weight streamed in two half-width pieces
            xq = st.tile([P, kt_in, S_LOC], BF16, tag="x", name="xq")
            nc.sync.dma_start(xq[:], qT_d.ap().rearrange(
                "p (kt s) -> p kt s", s=NQ))
            for piece in range(2):
                wq_sb = st.tile([P, kt_in, S_LOC], BF16, tag="x",
                                name="wq_sb")[:, :, :384]
                nc.scalar.dma_start(
                    wq_sb[:],
                    wqT_d.ap().rearrange("p (kt f) -> p kt f", f=D)
                    [:, :, piece * 384:(piece + 1) * 384])
                for fh in range(3):
                    ft = piece * 3 + fh
                    ps = psA.tile([P, EXP_BATCH * P], F32, tag="scores")
                    for kt in range(kt_in):
                        nc.tensor.matmul(
                            ps[:, :NQ],
                            wq_sb[:, kt, fh * P:(fh + 1) * P],
                            xq[:, kt, :],
                            start=(kt == 0), stop=(kt == kt_in - 1))
                    nc.vector.tensor_copy(qhT[:, ft, :], ps[:, :NQ])

        if DEBUG:
            nc.sync.dma_start(dbg["qhT"].ap(), qhT[:])
            nc.sync.dma_start(dbg["khT"].ap(), khT[:])
            nc.sync.dma_start(dbg["vh"].ap(), vh[:])

        # ---------- attention ----------
        if has_bias:
            nc.vector.memset(aT[:, H, :], 0.0)
            nc.vector.memset(aT[0:1, H, :], 1.0)

        cid = nc.partition_id()

        with tc.tile_pool(name="expp", bufs=2) as expp, \
             tc.tile_pool(name="mskp", bufs=1) as mskp, \
             tc.tile_pool(name="smp", bufs=4) as smp, \
             tc.tile_pool(name="wop", bufs=1) as wop, \
             tc.tile_pool(name="outp", bufs=2) as outp:
            tri = mskp.tile([P, P], BF16, tag="tri")
            nc.scalar.dma_start(tri[:], mask_d.ap())
            wo_sb = wop.tile([DH, ho, D], BF16, tag="wo")
            nc.scalar.dma_start(wo_sb[:], woT_d.ap().rearrange(
                "p (h f) -> p h f", f=D))
            for j in range(N_SLOTS):
                nkt = SLOT_NKT[j]
                # this core's diagonal tile position within the slot
                tri_start = cid * P + (8 * j) * P
                tail_start = cid * P + (8 * j + 1) * P
                W = (SLOT_NKT[-1] + 7) * P      # per-head exp region
                PB = 6                           # kt per pair-batch
                for hp in range(H // 2):
                    he, hu = 2 * hp, 2 * hp + 1
                    ft = hp
                    expb = expp.tile([P, 2 * W], BF16, tag="expb")
                    done = 0
                    while done < nkt:
                        nb = min(PB, nkt - done)
                        ps = psA.tile([P, 2 * PB * P], F32, tag="scores")
                        for b in range(nb):
                            kt = done + b
                            # even head on PE rows 0-63, odd on 64-127:
                            # adjacent issues run concurrently
                            nc.tensor.matmul(
                                ps[:, b * P:(b + 1) * P],
                                khT[0:DH, ft, kt * P:(kt + 1) * P],
                                qhT[0:DH, ft, j * P:(j + 1) * P],
                                start=True, stop=True)
                            nc.tensor.matmul(
                                ps[:, (PB + b) * P:(PB + b + 1) * P],
                                khT[DH:P, ft, kt * P:(kt + 1) * P],
                                qhT[DH:P, ft, j * P:(j + 1) * P],
                                start=True, stop=True)
                        # one strided exp over both heads' sub-batches
                        nc.scalar.activation(
                            expb[:].rearrange("p (g w) -> p g w", g=2)
                            [:, :, done * P:(done + nb) * P],
                            ps[:].rearrange("p (g w) -> p g w", g=2)
                            [:, :, :nb * P],
                            mybir.ActivationFunctionType.Exp,
                            scale=0.125)
                        done += nb
                    for h, base in ((he, 0), (hu, W)):
                        # causal mask on the diagonal tile + zero the
                        # beyond-causal tail (register-offset slices)
                        nc.vector.tensor_mul(
                            expb[:, bass.ds(base + tri_start, P)],
                            expb[:, bass.ds(base + tri_start, P)],
                            tri[:, :])
                        nc.vector.memset(
                            expb[:, bass.ds(base + tail_start, 7 * P)], 0.0)
                        # PV with ones-column -> rows 0..63, row 64 = den
                        po = psB.tile([P, P], F32, tag="pv")
                        for kt in range(nkt):
                            nc.tensor.matmul(
                                po[:VW, :],
                                vh[:, kt, h * VW:(h + 1) * VW],
                                expb[:, base + kt * P:base + (kt + 1) * P],
                                start=(kt == 0), stop=(kt == nkt - 1))
                        rec = smp.tile([P, P], F32, tag="rec")
                        nc.vector.tensor_copy(rec[DH:DH + 1, :],
                                              po[DH:DH + 1, :])
                        # hop partition 64 -> 0 (cross-partition ops read
                        # partition 0), then invert at partition 0
                        nc.sync.dma_start(rec[0:1, :], rec[DH:DH + 1, :])
                        nc.vector.reciprocal_approx_fast(rec[0:1, :],
                                                         rec[0:1, :])
                        bc = smp.tile([P, P], F32, tag="bc")
                        nc.gpsimd.partition_broadcast(bc[:DH, :],
                                                      rec[0:1, :])
                        nc.vector.tensor_mul(
                            aT[:, h, j * P:(j + 1) * P],
                            po[:DH, :], bc[:DH, :])

                # ---- output projection for this slot (overlaps later
                # slots' attention); K=64 per-head contraction ----
                osb = outp.tile([P, D], F32, tag="osb")
                for half in range(2):
                    pw = psB.tile([P, 384], F32, tag="pv")
                    for h in range(ho):
                        nc.tensor.matmul(
                            pw[:, :384],
                            aT[:, h, j * P:(j + 1) * P],
                            wo_sb[:, h, half * 384:(half + 1) * 384],
                            start=(h == 0), stop=(h == ho - 1))
                    nc.vector.tensor_copy(osb[:, half * 384:(half + 1) * 384],
                                          pw[:, :384])
                nc.sync.dma_start(out_d[j * P:(j + 1) * P, :], osb[:])

            if DEBUG:
                nc.sync.dma_start(dbg["aT"].ap(), aT[:, :H, :])


# ------------------------------------------------------------------
# host side
# ------------------------------------------------------------------

_CACHE = {}


def _get_nc(has_bias):
    key = has_bias
    if key not in _CACHE:
        _CACHE[key] = build(has_bias)
    return _CACHE[key]


def _bf16(x):
    return np.asarray(x, dtype=ml_dtypes.bfloat16)


def _build_mask(core):
    # static: valid iff key_row <= q_col
    return _bf16(np.tril(np.ones((P, P), dtype=np.float32)).T)


def kernel(q, k, v, wq, bq, wk, bk, wv, bv, wo, bo):
    q = np.asarray(q); k = np.asarray(k); v = np.asarray(v)
    wq = np.asarray(wq); wk = np.asarray(wk); wv = np.asarray(wv)
    wo = np.asarray(wo)
    bq = np.asarray(bq); bk = np.asarray(bk); bv = np.asarray(bv)
    bo = np.asarray(bo)
    has_bias = any(np.any(b) for b in (bq, bk, bv, bo))
    kt_in = 7 if has_bias else 6
    d_in = kt_in * P

    nc = _get_nc(has_bias)

    def _fold(x2d):
        # [kt_in*128, n] -> [128, kt_in*n] partition-major contiguous
        n = x2d.shape[1]
        return np.ascontiguousarray(
            x2d.reshape(kt_in, P, n).transpose(1, 0, 2).reshape(P, kt_in * n))

    def aug(xT, bias_row):
        # [768, n] -> folded [128, kt_in*n] with ones row at 768 (inputs)
        if not has_bias:
            return _bf16(_fold(xT))
        out = np.zeros((d_in, xT.shape[1]), dtype=np.float32)
        out[:D] = xT
        out[D] = bias_row
        return _bf16(_fold(out))

    def augw(w, b):
        # torch Linear weight [out, in] -> folded lhsT with bias row
        wT = w.T.astype(np.float32)
        if not has_bias:
            return _bf16(_fold(wT))
        out = np.zeros((d_in, D), dtype=np.float32)
        out[:D] = wT
        out[D] = b
        return _bf16(_fold(out))

    wqT = augw(wq, bq); wkT = augw(wk, bk); wvT = augw(wv, bv)
    ho = H + 1 if has_bias else H
    if has_bias:
        woT = np.zeros((ho * DH, D), dtype=np.float32)
        woT[:D] = wo.T
        woT[D] = bo
    else:
        woT = wo.T.astype(np.float32)
    woT = _bf16(np.ascontiguousarray(
        woT.reshape(ho, DH, D).transpose(1, 0, 2).reshape(DH, ho * D)))

    q2 = q[0].astype(np.float32)   # [S, D]
    k2 = k[0].astype(np.float32)
    v2 = v[0].astype(np.float32)

    in_maps = []
    for c in range(N_CORES):
        rows = np.concatenate(
            [np.arange(t * P, (t + 1) * P) for t in q_tiles_of_core(c)])
        qT = aug(q2[rows].T, 1.0)
        kT = aug(k2.T, 1.0)
        vT = aug(v2.T, 1.0)
        in_maps.append({
            "qT": qT, "kT": kT, "vT": vT,
            "wqT": wqT, "wkT": wkT, "wvT": wvT, "woT": woT,
            "mask": _build_mask(c),
        })

    res = bass_utils.run_bass_kernel_spmd(
        nc, in_maps, core_ids=list(range(N_CORES)))
    kernel.last_exec_time_ns = res.exec_time_ns

    out = np.empty((S, D), dtype=np.float32)
    for c in range(N_CORES):
        for j, t in enumerate(q_tiles_of_core(c)):
            out[t * P:(t + 1) * P] = res.results[c]["out"][j * P:(j + 1) * P]
    return out.reshape(1, S, D)
